# revision 1
# baseline (speedup 1.0000x reference)
"""Baichuan attention (B=2, S=2048, H=4096, 32 heads x 128) on 8 TRN2 NeuronCores.

Sharding: tensor-parallel over heads (4 heads per core), as in the original
model: W_pack column-sharded (per-head rows), o_proj row-sharded with the
partial-sum reduction done on the host during unshard ("all-reduce" of the
row-parallel output).

Per-core pipeline (all matmuls in float32r = full-rate fp32 on the PE):
  1. QKV projection from x^T (feature-major):
       qT,kT  [d, s] feature-major   (for the scores matmul)
       v      [k-tile, d] tile-major (for the context matmul)
     RoPE applied to qT/kT on the fly (partition-swap via SBUF-SBUF DMA).
  2. Per batch, heads-inner causal flash attention in transposed layout:
       sT[k, q] scores per head (PSUM), additive causal mask on diagonal
       blocks, Exp on ScalarE, denominator accumulated on VectorE and
       partition-reduced on GpSimd (broadcast form), ctx^T[d, q] accumulated
       on the PE, normalization on VectorE.
  3. o_proj: out[s, o] partial = ctx^T-stationary matmul against w_o^T shard.
Host: shards/transposes inputs (layout prep), sums the 8 row-parallel
partials into the full output.
"""
import os
import sys

for _p in ("/opt/trn_rl_repo", "/root/.axon_site/_ro/trn_rl_repo"):
    if os.path.isdir(_p) and _p not in sys.path:
        sys.path.insert(0, _p)

from contextlib import ExitStack

import ml_dtypes
import numpy as np

import concourse.bass as bass
import concourse.tile as tile
from concourse import bacc, bass_isa, mybir
from concourse.bass_utils import run_bass_kernel_spmd

F32 = mybir.dt.float32
F32R = mybir.dt.float32r
BF16 = mybir.dt.bfloat16

B, S, H = 2, 2048, 4096
NH, HD = 32, 128
NCORES = 8
HPC = NH // NCORES          # heads per core = 4
DPC = HPC * HD              # dims per core = 512
ROPE_BASE = 10000.0
NEG = -1.0e30

SBLK = 1024                 # projection s-block
NSB = S // SBLK             # 2 s-blocks per batch
QC = 512                    # attention q-chunk
NQC = S // QC               # 4 q-chunks
NHT = H // 128              # 32 h-tiles (contraction tiles)
NKT = S // 128              # 16 k-tiles per sequence


def _build():
    nc = bacc.Bacc("TRN2", target_bir_lowering=False, debug=False,
                   num_devices=NCORES)

    xT = nc.dram_tensor("xT", [B, H, S], BF16, kind="ExternalInput").ap()
    wqkT = nc.dram_tensor("wqkT", [H, 2 * DPC], BF16, kind="ExternalInput").ap()
    wvT = nc.dram_tensor("wvT", [H, DPC], BF16, kind="ExternalInput").ap()
    woT = nc.dram_tensor("woT", [DPC, H], F32R, kind="ExternalInput").ap()
    cosT = nc.dram_tensor("cosT", [HD, S], F32, kind="ExternalInput").ap()
    sinTm = nc.dram_tensor("sinTm", [HD, S], F32, kind="ExternalInput").ap()
    masks = nc.dram_tensor("masks", [4, 128, QC], F32, kind="ExternalInput").ap()

    qkT_s = nc.dram_tensor("qkT_s", [B, 2 * DPC, S], F32R).ap()
    v_s = nc.dram_tensor("v_s", [B, NKT, 128, DPC], F32R).ap()

    out = nc.dram_tensor("out", [B, S, H], F32, kind="ExternalOutput").ap()

    with tile.TileContext(nc) as tc, ExitStack() as top:
        # ---------------- Phase 1: QKV projection + rope ----------------
        with ExitStack() as ctx:
            singles = ctx.enter_context(tc.tile_pool(name="pj_singles", bufs=1))
            xpool = ctx.enter_context(tc.tile_pool(name="xslab", bufs=NHT))
            wpool = ctx.enter_context(tc.tile_pool(name="wslab", bufs=6))
            rpool = ctx.enter_context(tc.tile_pool(name="rope", bufs=6))
            opool = ctx.enter_context(tc.tile_pool(name="pj_out", bufs=6))
            pp = ctx.enter_context(tc.tile_pool(name="pj_psum", bufs=8,
                                                space="PSUM"))

            cos_sb = singles.tile([HD, S], F32)
            sin_sb = singles.tile([HD, S], F32)
            nc.sync.dma_start(out=cos_sb[:], in_=cosT[:])
            nc.sync.dma_start(out=sin_sb[:], in_=sinTm[:])

            def drain_qk(ps, b, row0, s0):
                """rope(psum tile [128, 512]) -> qkT_s[b, row0:row0+128, s0:s0+512]"""
                sl = ps[:]  # [128, 512]
                cslice = cos_sb[:, s0:s0 + 512]
                mslice = sin_sb[:, s0:s0 + 512]
                t1 = rpool.tile([128, 512], F32, tag="t1")
                nc.vector.tensor_mul(t1[:], sl, cslice)
                qsb = rpool.tile([128, 512], F32, tag="qsb")
                nc.scalar.copy(qsb[:], sl)
                qsw = rpool.tile([128, 512], F32, tag="qsw")
                nc.scalar.dma_start(out=qsw[0:64, :], in_=qsb[64:128, :])
                nc.scalar.dma_start(out=qsw[64:128, :], in_=qsb[0:64, :])
                t2 = rpool.tile([128, 512], F32, tag="t2")
                nc.vector.tensor_mul(t2[:], qsw[:], mslice)
                qo = opool.tile([128, 512], F32R, tag="qo")
                nc.vector.tensor_add(qo[:], t1[:], t2[:])
                nc.scalar.dma_start(out=qkT_s[b, row0:row0 + 128, s0:s0 + 512],
                                    in_=qo[:])

            for b in range(B):
                for sb in range(NSB):
                    s0 = sb * SBLK
                    xsl = [None] * NHT

                    # pass Q then pass K (stationary = weight tile)
                    for qk in range(2):
                        ps = [[pp.tile([128, 512], F32, tag="pp",
                                       name="pp")
                               for _ in range(2)] for _ in range(HPC)]
                        for h in range(NHT):
                            w = wpool.tile([128, DPC], BF16, tag="w")
                            nc.sync.dma_start(
                                out=w[:], in_=wqkT[h * 128:(h + 1) * 128,
                                                   qk * DPC:(qk + 1) * DPC])
                            if qk == 0:
                                xs = xpool.tile([128, SBLK], BF16, tag="xs",
                                                name="xs")
                                nc.sync.dma_start(
                                    out=xs[:], in_=xT[b, h * 128:(h + 1) * 128,
                                                      s0:s0 + SBLK])
                                xsl[h] = xs
                            for dt in range(HPC):
                                for sh in range(2):
                                    nc.tensor.matmul(
                                        ps[dt][sh][:],
                                        w[:, dt * 128:(dt + 1) * 128],
                                        xsl[h][:, sh * 512:(sh + 1) * 512],
                                        start=(h == 0), stop=(h == NHT - 1))
                        for dt in range(HPC):
                            for sh in range(2):
                                drain_qk(ps[dt][sh], b,
                                         qk * DPC + dt * 128,
                                         s0 + sh * 512)

                    # pass V (stationary = x tile), output [s, d] tile-major
                    psv = [pp.tile([128, DPC], F32, tag="pp", name="ppv")
                           for _ in range(SBLK // 128)]
                    for h in range(NHT):
                        w = wpool.tile([128, DPC], BF16, tag="w")
                        nc.sync.dma_start(
                            out=w[:], in_=wvT[h * 128:(h + 1) * 128, :])
                        for st in range(SBLK // 128):
                            nc.tensor.matmul(
                                psv[st][:],
                                xsl[h][:, st * 128:(st + 1) * 128],
                                w[:],
                                start=(h == 0), stop=(h == NHT - 1))
                    for st in range(SBLK // 128):
                        vo = opool.tile([128, DPC], F32R, tag="vo")
                        nc.vector.tensor_copy(vo[:], psv[st][:])
                        nc.scalar.dma_start(
                            out=v_s[b, (s0 + st * 128) // 128],
                            in_=vo[:])

        # ---------------- Phase 2+3: attention + o_proj ----------------
        with ExitStack() as ctx:
            singles = ctx.enter_context(tc.tile_pool(name="at_singles", bufs=1))
            qkvpool = ctx.enter_context(tc.tile_pool(name="at_qkv", bufs=1))
            prpool = ctx.enter_context(tc.tile_pool(name="at_pr", bufs=5))
            dpool = ctx.enter_context(tc.tile_pool(name="at_den", bufs=4))
            ctxpool = ctx.enter_context(tc.tile_pool(name="at_ctx", bufs=1))
            smpool = ctx.enter_context(tc.tile_pool(name="at_sm", bufs=2))
            wopool = ctx.enter_context(tc.tile_pool(name="at_wo", bufs=3))
            oopool = ctx.enter_context(tc.tile_pool(name="at_oo", bufs=5))
            ps_s = ctx.enter_context(tc.tile_pool(name="ps_s", bufs=4,
                                                  space="PSUM"))
            ps_c = ctx.enter_context(tc.tile_pool(name="ps_c", bufs=4,
                                                  space="PSUM"))

            mask_sb = singles.tile([128, 4, QC], F32)
            for dj in range(4):
                nc.sync.dma_start(out=mask_sb[:, dj, :], in_=masks[dj])

            for b in range(B):
                qT_all = qkvpool.tile([128, HPC, S], F32R, tag="qT")
                nc.sync.dma_start(
                    out=qT_all[:],
                    in_=qkT_s[b, 0:DPC, :].rearrange("(h p) s -> p h s", p=128))
                kT_all = qkvpool.tile([128, HPC, S], F32R, tag="kT")
                nc.sync.dma_start(
                    out=kT_all[:],
                    in_=qkT_s[b, DPC:2 * DPC, :].rearrange(
                        "(h p) s -> p h s", p=128))
                v_sb = qkvpool.tile([128, NKT, DPC], F32R, tag="v")
                nc.sync.dma_start(
                    out=v_sb[:], in_=v_s[b].rearrange("k p d -> p k d"))

                ctx_sb = ctxpool.tile([128, HPC, S], F32R, tag="ctx")

                for qc in range(NQC):
                    q0 = qc * QC
                    nkt = 4 * qc + 4
                    pc = [ps_c.tile([128, QC], F32, tag="pc", name="pc")
                          for _ in range(HPC)]
                    den = [dpool.tile([128, QC], F32, tag="den", name="den")
                           for _ in range(HPC)]
                    for kt in range(nkt):
                        pss = [ps_s.tile([128, QC], F32, tag="pss", name="pss")
                               for _ in range(HPC)]
                        for h in range(HPC):
                            nc.tensor.matmul(
                                pss[h][:],
                                kT_all[:, h, kt * 128:(kt + 1) * 128],
                                qT_all[:, h, q0:q0 + QC],
                                start=True, stop=True)
                        if kt >= 4 * qc:
                            for h in range(HPC):
                                nc.vector.tensor_add(
                                    pss[h][:], pss[h][:],
                                    mask_sb[:, kt - 4 * qc, :])
                        prt = []
                        for h in range(HPC):
                            pr = prpool.tile([128, QC], F32R, tag="pr",
                                             name="pr")
                            nc.scalar.activation(
                                out=pr[:], in_=pss[h][:],
                                func=mybir.ActivationFunctionType.Exp)
                            prt.append(pr)
                        for h in range(HPC):
                            nc.tensor.matmul(
                                pc[h][:],
                                v_sb[:, kt, h * HD:(h + 1) * HD],
                                prt[h][:],
                                start=(kt == 0), stop=(kt == nkt - 1))
                        for h in range(HPC):
                            if kt == 0:
                                nc.vector.tensor_copy(den[h][:],
                                                      prt[h][:].bitcast(F32))
                            else:
                                nc.vector.tensor_add(den[h][:], den[h][:],
                                                     prt[h][:].bitcast(F32))
                    # drain ctx psum fast, then normalize off the PE
                    for h in range(HPC):
                        cu = smpool.tile([128, QC], F32, tag="cu")
                        nc.vector.tensor_copy(cu[:], pc[h][:])
                        ar = smpool.tile([128, QC], F32, tag="ar")
                        nc.gpsimd.partition_all_reduce(
                            ar[:], den[h][:], channels=128,
                            reduce_op=bass_isa.ReduceOp.add)
                        rec1 = smpool.tile([1, QC], F32, tag="rec1")
                        nc.vector.reciprocal(rec1[:], ar[0:1, :])
                        rec = smpool.tile([128, QC], F32, tag="rec")
                        nc.gpsimd.partition_broadcast(rec[:], rec1[:])
                        nc.vector.tensor_mul(ctx_sb[:, h, q0:q0 + QC],
                                             cu[:], rec[:])

                # o_proj for batch b (w_o^T streamed per output chunk)
                for oc in range(H // 512):
                    wos = wopool.tile([128, HPC, 512], F32R, tag="wos")
                    nc.scalar.dma_start(
                        out=wos[:],
                        in_=woT[:, oc * 512:(oc + 1) * 512].rearrange(
                            "(h p) o -> p h o", p=128))
                    for st in range(S // 128):
                        po = ps_s.tile([128, 512], F32, tag="pss", name="po")
                        for h in range(HPC):
                            nc.tensor.matmul(
                                po[:],
                                ctx_sb[:, h, st * 128:(st + 1) * 128],
                                wos[:, h, :],
                                start=(h == 0), stop=(h == HPC - 1))
                        ot = oopool.tile([128, 512], F32, tag="ot")
                        if st % 2 == 0:
                            nc.scalar.copy(ot[:], po[:])
                        else:
                            nc.vector.tensor_copy(ot[:], po[:])
                        nc.scalar.dma_start(
                            out=out[b, st * 128:(st + 1) * 128,
                                    oc * 512:(oc + 1) * 512],
                            in_=ot[:])

    nc.compile()
    return nc


_CACHE = {}


def _host_prep(x, w_pack, w_o):
    """Build per-core input maps (sharding + layout prep)."""
    x = np.asarray(x, dtype=np.float32)
    w_pack = np.asarray(w_pack, dtype=np.float32)
    w_o = np.asarray(w_o, dtype=np.float32)

    xT = np.ascontiguousarray(
        x.transpose(0, 2, 1).astype(ml_dtypes.bfloat16))   # [B, H, S] bf16

    inv_freq = 1.0 / (ROPE_BASE ** (np.arange(0, HD, 2, dtype=np.float32) / HD))
    t = np.arange(S, dtype=np.float32)
    freqs = np.outer(t, inv_freq)                            # [S, HD/2]
    emb = np.concatenate([freqs, freqs], axis=-1)            # [S, HD]
    cosT = np.ascontiguousarray(np.cos(emb).T, dtype=np.float32)   # [HD, S]
    sinT = np.sin(emb).T.astype(np.float32)
    sinTm = np.concatenate([-sinT[:HD // 2], sinT[HD // 2:]], axis=0)
    sinTm = np.ascontiguousarray(sinTm)

    kk = np.arange(128)[:, None]
    qq = np.arange(QC)[None, :]
    masks = np.stack([
        np.where(kk + 128 * dj <= qq, 0.0, NEG).astype(np.float32)
        for dj in range(4)
    ])                                                        # [4, 128, QC]

    scale = float(HD) ** -0.5
    in_maps = []
    for c in range(NCORES):
        r0 = c * DPC
        wq = w_pack[r0:r0 + DPC, :] * scale                   # [512, H]
        wk = w_pack[H + r0:H + r0 + DPC, :]
        wv = w_pack[2 * H + r0:2 * H + r0 + DPC, :]
        wqkT = np.ascontiguousarray(np.concatenate(
            [wq.T, wk.T], axis=1).astype(ml_dtypes.bfloat16))  # [H, 1024]
        wvT = np.ascontiguousarray(wv.T.astype(ml_dtypes.bfloat16))
        woT = np.ascontiguousarray(w_o[:, r0:r0 + DPC].T)     # [512, H]
        in_maps.append({
            "xT": xT, "wqkT": wqkT, "wvT": wvT, "woT": woT,
            "cosT": cosT, "sinTm": sinTm, "masks": masks,
        })
    return in_maps


def kernel(x, w_pack, w_o, _trace=False, _trace_kwargs=None):
    if "nc" not in _CACHE:
        _CACHE["nc"] = _build()
    nc = _CACHE["nc"]

    in_maps = _host_prep(x, w_pack, w_o)
    res = run_bass_kernel_spmd(nc, in_maps, list(range(NCORES)),
                               trace=_trace, **(_trace_kwargs or {}))
    acc = res.results[0]["out"].astype(np.float32)
    for c in range(1, NCORES):
        acc = acc + res.results[c]["out"]
    if _trace:
        kernel.last_results = res
    return acc



# revision 12
# speedup vs baseline: 1.1338x; 1.1338x over previous
"""Baichuan attention (B=2, S=2048, H=4096, 32 heads x 128) on 8 TRN2 NeuronCores.

Tensor-parallel over heads (4 per core); o_proj row-parallel with the
partial-sum reduction done on host during unshard.

Per-core pipeline, all in bf16 matmuls (fp32 PSUM accumulate):
  Per batch b (sequential phases, all intermediates SBUF-resident):
    proj(b):  per 512-col s-chunk: Q/K/V projections; RoPE applied on the
              Q/K drains (ACT copy + partition-swap DMA + 3 bf16 DVE ops),
              results written to persistent bf16 SBUF tiles qT/kT [d,h,s]
              and v [k, kt, d].
    attn(b):  per 512-q chunk, per 128-k block, per head:
              scores MM (bf16, PSUM f32) -> causal mask on diagonal blocks
              via vector.tensor_mask_reduce -> Exp on ACT (bf16 out) ->
              ctx MM accumulate + den MM (one-hot lhsT -> per-head row of a
              shared [4,512] PSUM den tile; PSUM accumulates across k).
              Then one reciprocal_approx_fast per q-chunk, GpSimd
              partition-broadcast per head, fused normalize-to-bf16 drain.
    oproj(b): w_o bf16 streamed per 512-col out-chunk, ctx-stationary MMs,
              drains alternate Scalar/Vector, direct DMA to DRAM out.
Host: shards/transposes inputs, sums the 8 row-parallel partials.
"""
import os
import sys

for _p in ("/opt/trn_rl_repo", "/root/.axon_site/_ro/trn_rl_repo"):
    if os.path.isdir(_p) and _p not in sys.path:
        sys.path.insert(0, _p)

from contextlib import ExitStack

import ml_dtypes
import numpy as np

import concourse.bass as bass
import concourse.tile as tile
from concourse import bacc, mybir
from concourse.bass_utils import run_bass_kernel_spmd

F32 = mybir.dt.float32
BF16 = mybir.dt.bfloat16

B, S, H = 2, 2048, 4096
NH, HD = 32, 128
NCORES = 8
HPC = NH // NCORES          # heads per core = 4
DPC = HPC * HD              # dims per core = 512
ROPE_BASE = 10000.0

SBLK = 512                  # projection s-chunk
NSB = S // SBLK             # 4 s-chunks per batch
QC = 512                    # attention q-chunk
NQC = S // QC               # 4 q-chunks
NHT = H // 128              # 32 contraction tiles
NKT = S // 128              # 16 k-blocks per sequence
EXPF = mybir.ActivationFunctionType.Exp


def _build():
    nc = bacc.Bacc("TRN2", target_bir_lowering=False, debug=False,
                   num_devices=NCORES)

    xT = nc.dram_tensor("xT", [B, NHT, 128, S], BF16, kind="ExternalInput").ap()
    # wqkT[qk, dt, p, h*128+d] = w_{q|k}^T[128*h + p, 128*dt + d]
    wqkT = nc.dram_tensor("wqkT", [2, HPC, 128, NHT * 128], BF16,
                          kind="ExternalInput").ap()
    # wvT[p, h, d] = w_v^T[128*h + p, d]
    wvT = nc.dram_tensor("wvT", [128, NHT, DPC], BF16,
                         kind="ExternalInput").ap()
    woT = nc.dram_tensor("woT", [DPC, H], BF16, kind="ExternalInput").ap()
    cosT = nc.dram_tensor("cosT", [HD, S], BF16, kind="ExternalInput").ap()
    sinTm = nc.dram_tensor("sinTm", [HD, S], BF16, kind="ExternalInput").ap()
    # iotas[:, i] = k + 128*i (mask_start per diag block), iotas[:, 4] = 512.0
    iotas = nc.dram_tensor("iotas", [128, 5], F32, kind="ExternalInput").ap()
    # id16[k, 4h+j] = 1 if j == h else 0  (den-matmul one-hot lhsT)
    id16 = nc.dram_tensor("id16", [128, 16], BF16, kind="ExternalInput").ap()
    masks = nc.dram_tensor("masks", [4, 128, QC], F32, kind="ExternalInput").ap()

    out = nc.dram_tensor("out", [B, S, H], F32, kind="ExternalOutput").ap()

    with tile.TileContext(nc) as tc, ExitStack() as top:
        persist = top.enter_context(tc.tile_pool(name="persist", bufs=1))

        cos_sb = persist.tile([HD, S], BF16)
        sin_sb = persist.tile([HD, S], BF16)
        iota_sb = persist.tile([128, 5], F32)
        id16_sb = persist.tile([128, 16], BF16)
        mask_sb = persist.tile([128, 4, QC], F32)
        nc.sync.dma_start(out=cos_sb[:], in_=cosT[:])
        nc.sync.dma_start(out=sin_sb[:], in_=sinTm[:])
        nc.sync.dma_start(out=iota_sb[:], in_=iotas[:])
        nc.sync.dma_start(out=id16_sb[:], in_=id16[:])
        for dj in range(4):
            nc.sync.dma_start(out=mask_sb[:, dj, :], in_=masks[dj])

        qT = persist.tile([128, HPC, S], BF16, tag="qT")
        kT = persist.tile([128, HPC, S], BF16, tag="kT")
        v_sb = persist.tile([128, NKT, DPC], BF16, tag="v")
        ctx_sb = persist.tile([128, HPC, S], BF16, tag="ctx")

        for b in range(B):
            # ---------------- proj(b) ----------------
            with ExitStack() as ctx:
                xpool = ctx.enter_context(tc.tile_pool(name="xslab", bufs=NHT))
                wpool = ctx.enter_context(tc.tile_pool(name="wslab", bufs=6))
                rpool = ctx.enter_context(tc.tile_pool(name="rope", bufs=4))
                pp = ctx.enter_context(tc.tile_pool(name="pj_psum", bufs=8,
                                                    space="PSUM"))

                wvs = wpool.tile([128, NHT, DPC], BF16, tag="wv", bufs=1)
                nc.sync.dma_start(out=wvs[:], in_=wvT[:])

                for sb in range(NSB):
                    s0 = sb * SBLK
                    xsl = []
                    for h in range(NHT):
                        xs = xpool.tile([128, SBLK], BF16, tag="xs")
                        nc.sync.dma_start(out=xs[:],
                                          in_=xT[b, h, :, s0:s0 + SBLK])
                        xsl.append(xs)

                    # Q and K passes: out [d(head dt), s] with rope on drain
                    for qk in range(2):
                        for dt in range(HPC):
                            ps = pp.tile([128, SBLK], F32, tag="pp",
                                         name=f"pj{qk}{dt}")
                            w = wpool.tile([128, NHT, 128], BF16, tag="w", bufs=3)
                            nc.sync.dma_start(out=w[:], in_=wqkT[qk, dt])
                            for h in range(NHT):
                                nc.tensor.matmul(
                                    ps[:], w[:, h, :], xsl[h][:],
                                    start=(h == 0), stop=(h == NHT - 1))
                            # rope drain -> (qT|kT)[:, dt, s0:s0+SBLK]
                            dst = (qT if qk == 0 else kT)[:, dt, s0:s0 + SBLK]
                            qsb = rpool.tile([128, SBLK], BF16, tag="qsb")
                            nc.scalar.copy(qsb[:], ps[:])
                            qsw = rpool.tile([128, SBLK], BF16, tag="qsw")
                            nc.scalar.dma_start(out=qsw[0:64, :],
                                                in_=qsb[64:128, :])
                            nc.scalar.dma_start(out=qsw[64:128, :],
                                                in_=qsb[0:64, :])
                            t1 = rpool.tile([128, SBLK], BF16, tag="t1")
                            nc.vector.tensor_mul(t1[:], qsb[:],
                                                 cos_sb[:, s0:s0 + SBLK])
                            t2 = rpool.tile([128, SBLK], BF16, tag="t2")
                            nc.vector.tensor_mul(t2[:], qsw[:],
                                                 sin_sb[:, s0:s0 + SBLK])
                            nc.vector.tensor_add(dst, t1[:], t2[:])

                    # V pass: out [s-tile, d] tile-major
                    for st in range(SBLK // 128):
                        psv = pp.tile([128, DPC], F32, tag="pp",
                                      name=f"pjv{st}")
                        for h in range(NHT):
                            nc.tensor.matmul(
                                psv[:],
                                xsl[h][:, st * 128:(st + 1) * 128],
                                wvs[:, h, :],
                                start=(h == 0), stop=(h == NHT - 1))
                        nc.vector.tensor_copy(
                            v_sb[:, (s0 + st * 128) // 128, :], psv[:])

            # ---------------- attn(b) ----------------
            with ExitStack() as ctx:
                prpool = ctx.enter_context(tc.tile_pool(name="at_pr", bufs=8))
                smpool = ctx.enter_context(tc.tile_pool(name="at_sm", bufs=6))
                ps_s = ctx.enter_context(tc.tile_pool(name="ps_s", bufs=3,
                                                      space="PSUM"))
                ps_c = ctx.enter_context(tc.tile_pool(name="ps_c", bufs=4,
                                                      space="PSUM"))
                ps_d = ctx.enter_context(tc.tile_pool(name="ps_d", bufs=1,
                                                      space="PSUM"))

                for qc in range(NQC):
                    q0 = qc * QC
                    nkt = 4 * qc + 4
                    pc = [ps_c.tile([128, QC], F32, tag="pc", name=f"pc{h}")
                          for h in range(HPC)]
                    pden = ps_d.tile([HPC, QC], F32, tag="pden", name="pden")
                    # two passes of 2 heads each (PSUM: 4 pc + 3 pss + 1 den)
                    for hp in range(2):
                        hs = (2 * hp, 2 * hp + 1)
                        for kt in range(nkt):
                            pss = {}
                            for h in hs:
                                pss[h] = ps_s.tile([128, QC], F32, tag="pss",
                                                   name=f"pss{h}")
                                nc.tensor.matmul(
                                    pss[h][:],
                                    kT[:, h, kt * 128:(kt + 1) * 128],
                                    qT[:, h, q0:q0 + QC],
                                    start=True, stop=True)
                            if kt >= 4 * qc:
                                i = kt - 4 * qc
                                for h in hs:
                                    nc.vector.tensor_add(
                                        pss[h][:], pss[h][:],
                                        mask_sb[:, i, :])
                            prt = {}
                            for h in hs:
                                pr = prpool.tile([128, QC], BF16, tag="pr",
                                                 name=f"pr{h}")
                                nc.scalar.activation(out=pr[:],
                                                     in_=pss[h][:],
                                                     func=EXPF)
                                prt[h] = pr
                            for h in hs:
                                nc.tensor.matmul(
                                    pc[h][:],
                                    v_sb[:, kt, h * HD:(h + 1) * HD],
                                    prt[h][:],
                                    start=(kt == 0), stop=(kt == nkt - 1))
                                nc.tensor.matmul(
                                    pden[0:HPC, :],
                                    id16_sb[:, 4 * h:4 * h + 4],
                                    prt[h][:],
                                    start=(hp == 0 and kt == 0 and h == hs[0]),
                                    stop=(hp == 1 and kt == nkt - 1
                                          and h == hs[1]))
                    rec = smpool.tile([HPC, QC], F32, tag="rec")
                    nc.vector.reciprocal(out=rec[:], in_=pden[0:HPC, :])
                    for h in range(HPC):
                        rh = smpool.tile([1, QC], F32, tag=f"rh{h}", bufs=2)
                        nc.scalar.dma_start(out=rh[:], in_=rec[h:h + 1, :])
                        rbc = smpool.tile([128, QC], F32, tag="rbc")
                        nc.gpsimd.partition_broadcast(rbc[:], rh[:])
                        nc.vector.tensor_mul(ctx_sb[:, h, q0:q0 + QC],
                                             pc[h][:], rbc[:])

            # ---------------- oproj(b) ----------------
            with ExitStack() as ctx:
                wopool = ctx.enter_context(tc.tile_pool(name="at_wo", bufs=3))
                oopool = ctx.enter_context(tc.tile_pool(name="at_oo", bufs=6))
                ps_o = ctx.enter_context(tc.tile_pool(name="ps_o", bufs=4,
                                                      space="PSUM"))
                for oc in range(H // 512):
                    wos = wopool.tile([128, HPC, 512], BF16, tag="wos")
                    nc.scalar.dma_start(
                        out=wos[:],
                        in_=woT[:, oc * 512:(oc + 1) * 512].rearrange(
                            "(h p) o -> p h o", p=128))
                    for st in range(S // 128):
                        po = ps_o.tile([128, 512], F32, tag="po", name="po")
                        for h in range(HPC):
                            nc.tensor.matmul(
                                po[:],
                                ctx_sb[:, h, st * 128:(st + 1) * 128],
                                wos[:, h, :],
                                start=(h == 0), stop=(h == HPC - 1))
                        ot = oopool.tile([128, 512], F32, tag="ot")
                        if st % 2 == 0:
                            nc.scalar.copy(ot[:], po[:])
                        else:
                            nc.vector.tensor_copy(ot[:], po[:])
                        nc.scalar.dma_start(
                            out=out[b, st * 128:(st + 1) * 128,
                                    oc * 512:(oc + 1) * 512],
                            in_=ot[:])

    nc.compile()
    return nc


_CACHE = {}


def _host_prep(x, w_pack, w_o):
    """Build per-core input maps (sharding + layout prep)."""
    x = np.asarray(x, dtype=np.float32)
    w_pack = np.asarray(w_pack, dtype=np.float32)
    w_o = np.asarray(w_o, dtype=np.float32)

    xT = np.ascontiguousarray(
        x.transpose(0, 2, 1).reshape(B, NHT, 128, S)
        .astype(ml_dtypes.bfloat16))                     # [B, 32, 128, S]

    inv_freq = 1.0 / (ROPE_BASE ** (np.arange(0, HD, 2, dtype=np.float32) / HD))
    t = np.arange(S, dtype=np.float32)
    freqs = np.outer(t, inv_freq)                            # [S, HD/2]
    emb = np.concatenate([freqs, freqs], axis=-1)            # [S, HD]
    cosT = np.ascontiguousarray(
        np.cos(emb).T.astype(ml_dtypes.bfloat16))            # [HD, S]
    sinT = np.sin(emb).T.astype(np.float32)
    sinTm = np.concatenate([-sinT[:HD // 2], sinT[HD // 2:]], axis=0)
    sinTm = np.ascontiguousarray(sinTm.astype(ml_dtypes.bfloat16))

    kk = np.arange(128, dtype=np.float32)
    iotas = np.stack([kk + 128 * i for i in range(4)]
                     + [np.full(128, 512.0, np.float32)], axis=1)
    iotas = np.ascontiguousarray(iotas)                      # [128, 5]

    kk2 = np.arange(128)[:, None]
    qq = np.arange(QC)[None, :]
    masks = np.stack([
        np.where(kk2 + 128 * dj <= qq, 0.0, -1.0e30).astype(np.float32)
        for dj in range(4)
    ])                                                       # [4, 128, QC]

    id16 = np.zeros((128, 16), dtype=np.float32)
    for h in range(4):
        id16[:, 4 * h + h] = 1.0
    id16 = np.ascontiguousarray(id16.astype(ml_dtypes.bfloat16))

    scale = float(HD) ** -0.5
    in_maps = []
    for c in range(NCORES):
        r0 = c * DPC
        wq = w_pack[r0:r0 + DPC, :] * scale                  # [512, H]
        wk = w_pack[H + r0:H + r0 + DPC, :]
        wv = w_pack[2 * H + r0:2 * H + r0 + DPC, :]
        # wqkT[qk, dt, p, 128h+d] = w^T[128h+p, 128dt+d]
        wqkT = np.stack([wq.T, wk.T], axis=0)                # [2, H, 512]
        wqkT = wqkT.reshape(2, NHT, 128, HPC, 128)           # [2,h,p,dt,d]
        wqkT = wqkT.transpose(0, 3, 2, 1, 4).reshape(2, HPC, 128, NHT * 128)
        wqkT = np.ascontiguousarray(wqkT.astype(ml_dtypes.bfloat16))
        # wvT[p, h, d] = w_v^T[128h+p, d]
        wvT = wv.T.reshape(NHT, 128, DPC).transpose(1, 0, 2)
        wvT = np.ascontiguousarray(wvT.astype(ml_dtypes.bfloat16))
        woT = np.ascontiguousarray(
            w_o[:, r0:r0 + DPC].T.astype(ml_dtypes.bfloat16))  # [512, H]
        in_maps.append({
            "xT": xT, "wqkT": wqkT, "wvT": wvT, "woT": woT,
            "cosT": cosT, "sinTm": sinTm, "iotas": iotas, "id16": id16,
            "masks": masks,
        })
    return in_maps


def kernel(x, w_pack, w_o, _trace=False, _trace_kwargs=None):
    if "nc" not in _CACHE:
        _CACHE["nc"] = _build()
    nc = _CACHE["nc"]

    in_maps = _host_prep(x, w_pack, w_o)
    res = run_bass_kernel_spmd(nc, in_maps, list(range(NCORES)),
                               trace=_trace, **(_trace_kwargs or {}))
    acc = res.results[0]["out"].astype(np.float32)
    for c in range(1, NCORES):
        acc = acc + res.results[c]["out"]
    if _trace:
        kernel.last_results = res
    return acc
